# revision 1
# baseline (speedup 1.0000x reference)
"""Trainium2 Bass kernel for nn_Attention_Module_15152644620833 (v3).

Reference computation (T=4096, B=8, D=1024, H=64, half=2048):
    q   = x[:half] @ Wq + bq            (half, B, H)
    k   = x @ Wk + bk                   (T, B, H)
    val = x @ Wv + bv                   (T, B, H)
    r   = posenc(T, D) @ Wr + br        (T, H)
    scores[b] = q[b] @ (k[b] + r).T + bias[b][None, :]
        where bias[b][m] = sum(u) * k[m,b,:].sum() + sum(v) * r[m,:].sum()
    causal mask on first `half` key positions, softmax over all T keys,
    out = attn @ val                    (half, B, H)

Sharding: data-parallel over batch, one batch per NeuronCore (8 cores).
Each core receives its batch slice pre-transposed (x.T, contiguous).  The
positional-encoding projection r (identical on all cores) is sharded: each
core computes a 512-key slice of r.T and the full r.T is AllGathered.

Per-core device algorithm (f32r matmuls, fp32 PSUM):
    K2 (128, T):  rows 0:64 = k.T + bk, rows 64:128 = r.T + br
    q2 (128, half): rows 0:64 = q.T + bq, rows 64:128 = the same q.T
        -> scoresT(m,t) = K2[:,mtile].T @ q2 = q.k + q.r   (K=128)
    softmax key bias folded multiplicatively into val:
        exp(s + bias[m]) = exp(s)*eb[m]; eb scales both the val columns and
        the ones column (denominator), so attn is unchanged (exact).
        bias[m] = K2[:,m].T @ [u_sum x64; v_sum x64]   (one N=1 matmul/tile)
    causal mask: accumulate identity.T @ maskA (-1e30) into scores PSUM of
        diagonal tiles; fully-masked tiles are skipped.
    expT = exp(scoresT)  (no max subtraction: |scores| < ~60, safe in f32)
    outT (65, 512) += valaug[mtile].T @ expT  per query chunk (col 64 of
        valaug = eb -> row 64 of outT = softmax denominator)
    out (128, 64) = transpose(outT) * (1/denominator)

Schedule: sweep 1 streams x.T and runs all gather-independent projections
while the AllGather is in flight (its DMAs ride the ACT HWDGE ring so they
cannot head-of-line-block the x.T stream on the SP ring); sweep 2 runs
attention query-chunk-outer with the attnval matmuls software-pipelined two
exp-groups behind the score matmuls.
"""

import math

import numpy as np

T, B, D, H = 4096, 8, 1024, 64
HALF = T // 2
P = 128
DC = D // P          # 8 d-chunks
NCH = T // 512       # 8 key chunks of 512
NTQ = HALF // 512    # 4 query chunks of 512
MT = T // P          # 32 key tiles of 128
NCORES = 8

_CACHE = {}


def _posenc_T():
    """Constant positional encoding, transposed to (D, T), float32."""
    pos = np.arange(T, dtype=np.float32)[:, None]
    div = np.exp(
        (np.arange(0, D, 2, dtype=np.float32)
         * np.float32(-(math.log(10000.0) / D))).astype(np.float32)
    ).astype(np.float32)
    ang = (pos * div).astype(np.float32)
    pe = np.stack([np.sin(ang), np.cos(ang)], axis=-1).reshape(T, D)
    return np.ascontiguousarray(pe.astype(np.float32).T)


def _live(tq, mt):
    """Key tile mt contributes to query chunk tq (not fully masked)."""
    m0 = mt * P
    return not (m0 >= tq * 512 + 512 and m0 < HALF)


def _is_diag(tq, mt):
    return tq * 512 <= mt * P < tq * 512 + 512


def _build_module():
    import concourse.bacc as bacc
    import concourse.bass_isa as bass_isa
    import concourse.mybir as mybir
    from concourse.masks import make_identity
    from concourse.tile import TileContext

    f32 = mybir.dt.float32
    f32r = mybir.dt.float32r
    Exp = mybir.ActivationFunctionType.Exp

    nc = bacc.Bacc(num_devices=NCORES)

    xT_h = nc.dram_tensor("xT", [D, T], f32r, kind="ExternalInput")
    peTs_h = nc.dram_tensor("peTs", [D, 512], f32r, kind="ExternalInput")
    wkv_h = nc.dram_tensor("wkv", [D, 2 * H], f32r, kind="ExternalInput")
    wqq_h = nc.dram_tensor("wqq", [D, 2 * H], f32r, kind="ExternalInput")
    wr2_h = nc.dram_tensor("wr2", [D, 2 * H], f32r, kind="ExternalInput")
    bkv_h = nc.dram_tensor("bkv", [2 * H, 1], f32, kind="ExternalInput")
    bqq_h = nc.dram_tensor("bqq", [2 * H, 1], f32, kind="ExternalInput")
    br_h = nc.dram_tensor("br", [H, 1], f32, kind="ExternalInput")
    u_h = nc.dram_tensor("u", [H, 1], f32, kind="ExternalInput")
    v_h = nc.dram_tensor("v", [H, 1], f32, kind="ExternalInput")
    out_h = nc.dram_tensor("out", [HALF, H], f32, kind="ExternalOutput")

    xT_r = xT_h[:, :].rearrange("(c p) t -> p c t", p=P)       # (128, 8, T)
    peTs_r = peTs_h[:, :].rearrange("(c p) t -> p c t", p=P)   # (128, 8, 512)
    wkv_r = wkv_h[:, :].rearrange("(c p) h -> p c h", p=P)
    wqq_r = wqq_h[:, :].rearrange("(c p) h -> p c h", p=P)
    wr2_r = wr2_h[:, :].rearrange("(c p) h -> p c h", p=P)
    out_r = out_h[:, :].rearrange("(g p) h -> p g h", p=P)     # (128, 16, 64)

    with TileContext(nc) as tc, tc.tile_pool(name="persist", bufs=1) as persist:

        def _tile(shape, name, dt=f32):
            return persist.tile(shape, dt, name=name)

        # ---- persistent SBUF tiles -------------------------------------
        wkv_sb = _tile([P, DC, 2 * H], "wkv_sb", f32r)
        wqq_sb = _tile([P, DC, 2 * H], "wqq_sb", f32r)
        wr2_sb = _tile([P, DC, 2 * H], "wr2_sb", f32r)
        bkv_sb = _tile([2 * H, 1], "bkv_sb")
        bqq_sb = _tile([2 * H, 1], "bqq_sb")
        brc_sb = _tile([2 * H, 1], "brc_sb")    # br loaded at rows 64:128
        u_cl = _tile([H, 1], "u_cl")
        v_cl = _tile([H, 1], "v_cl")
        u_all = _tile([H, 1], "u_all")
        v_all = _tile([H, 1], "v_all")
        uvf = _tile([2 * H, 1], "uvf")
        uv_col = _tile([2 * H, 4], "uv_col", f32r)
        id_sb = _tile([P, P], "id_sb", f32r)
        maskA = _tile([P, 4, 512], "maskA", f32r)
        K2 = _tile([P, T], "K2", f32r)          # 0:64 k.T+bk, 64:128 r.T+br
        vT = _tile([P, T], "vT", f32r)          # rows 64:128 = v.T+bv
        q2T = _tile([P, HALF], "q2T", f32r)     # rows 0:64 and 64:128 = q.T
        valaug = _tile([P, MT, H + 1], "valaug", f32r)
        ebias = _tile([P, MT], "ebias")
        outall = _tile([P, HALF // P, H], "outall")

        # ---- constants / small setup -----------------------------------
        nc.sync.dma_start(wr2_sb[:], wr2_r)
        nc.sync.dma_start(wkv_sb[:], wkv_r)
        nc.sync.dma_start(wqq_sb[:], wqq_r)
        nc.sync.dma_start(bkv_sb[:], bkv_h[:, :])
        nc.sync.dma_start(bqq_sb[:], bqq_h[:, :])
        nc.sync.dma_start(brc_sb[H : 2 * H, :], br_h[:, :])
        nc.sync.dma_start(u_cl[:], u_h[:, :])
        nc.sync.dma_start(v_cl[:], v_h[:, :])

        with (
            tc.tile_pool(name="xstream", bufs=2) as xpool,
            tc.tile_pool(name="dramp", bufs=1, space="DRAM") as dramp,
        ):
            # ---- r.T shard + AllGather kickoff (identical r everywhere) -
            with tc.tile_pool(name="ppj", bufs=1, space="PSUM") as ppj:
                pet = xpool.tile([P, DC, 512], f32r, name="pet", tag="xt")
                nc.sync.dma_start(pet[:], peTs_r)
                rp = ppj.tile([P, 512], f32, name="rp", tag="kv", bufs=2)
                for dc in range(DC):
                    nc.tensor.matmul(
                        rp[:], wr2_sb[:, dc, :], pet[:, dc, :],
                        start=(dc == 0), stop=(dc == DC - 1),
                    )
                rloc_sb = xpool.tile([P, 512], f32r, name="rloc_sb", tag="rloc")
                nc.vector.tensor_scalar_add(
                    rloc_sb[H:P, :], rp[H:P, :], brc_sb[H : 2 * H, :]
                )
                rloc_dr = dramp.tile([H, 512], f32r, name="rloc_dr")
                nc.scalar.dma_start(rloc_dr[:], rloc_sb[H:P, :])
                rTg_dr = dramp.tile([NCORES * H, 512], f32r, name="rTg_dr",
                                    addr_space="Shared")
                nc.gpsimd.collective_compute(
                    "AllGather", mybir.AluOpType.bypass,
                    replica_groups=[list(range(NCORES))],
                    ins=[rloc_dr[:]], outs=[rTg_dr[:]],
                )
                # on the ACT HWDGE ring: this DMA waits on the collective,
                # and on the SP ring it would head-of-line-block the x.T
                # chunk stream behind it
                nc.scalar.dma_start(
                    K2[H:P, :].rearrange("h (c m) -> h c m", c=NCH),
                    rTg_dr[:].rearrange("(c h) m -> h c m", h=H),
                )

                # f32r tiles cannot be produced by memset/affine_select directly
                # (ISA/verifier); build constants in f32 scratch, cast-copy on DVE.
                with tc.tile_pool(name="setupf", bufs=1) as setupf:
                    idf = setupf.tile([P, P], f32, name="idf")
                    make_identity(nc, idf[:])
                    nc.vector.tensor_copy(id_sb[:], idf[:])
                    maskAf = setupf.tile([P, 4, 512], f32, name="maskAf")
                    nc.gpsimd.memset(maskAf[:], 0.0)
                    for rel in range(4):
                        nc.gpsimd.affine_select(
                            out=maskAf[:, rel, :], in_=maskAf[:, rel, :],
                            compare_op=mybir.AluOpType.is_ge, fill=-1e30,
                            base=-P * rel, pattern=[[1, 512]], channel_multiplier=-1,
                        )
                    nc.vector.tensor_copy(maskA[:], maskAf[:])
                    onesf = setupf.tile([P, MT], f32, name="onesf")
                    nc.gpsimd.memset(onesf[:], 1.0)
                    nc.vector.tensor_copy(valaug[:, :, H], onesf[:, 0:MT])

                # sum(u) broadcast to rows 0:64, sum(v) to rows 64:128 of uv_col
                nc.gpsimd.partition_all_reduce(u_all[:], u_cl[:], H, bass_isa.ReduceOp.add)
                nc.gpsimd.partition_all_reduce(v_all[:], v_cl[:], H, bass_isa.ReduceOp.add)
                nc.vector.tensor_copy(uvf[0:H, :], u_all[:])
                nc.sync.dma_start(uvf[H : 2 * H, :], v_all[:])  # partition shift
                nc.vector.tensor_copy(uv_col[:], uvf[:, 0:1].to_broadcast((2 * H, 4)))


                # ---- sweep 1: gather-independent projections ------------
                for c in range(NCH):
                    sl = slice(c * 512, (c + 1) * 512)
                    xt = xpool.tile([P, DC, 512], f32r, name="xt", tag="xt")
                    nc.sync.dma_start(xt[:], xT_r[:, :, sl])

                    kvp = ppj.tile([P, 512], f32, name="kvp", tag="kv", bufs=2)
                    for dc in range(DC):
                        nc.tensor.matmul(
                            kvp[:], wkv_sb[:, dc, :], xt[:, dc, :],
                            start=(dc == 0), stop=(dc == DC - 1),
                        )
                    nc.vector.tensor_scalar_add(
                        K2[0:H, sl], kvp[0:H, :], bkv_sb[0:H, :]
                    )
                    nc.vector.tensor_scalar_add(
                        vT[H:P, sl], kvp[H:P, :], bkv_sb[H : 2 * H, :]
                    )

                    if c < NTQ:
                        qp = ppj.tile([P, 512], f32, name="qp", tag="kv", bufs=2)
                        for dc in range(DC):
                            nc.tensor.matmul(
                                qp[:], wqq_sb[:, dc, :], xt[:, dc, :],
                                start=(dc == 0), stop=(dc == DC - 1),
                            )
                        nc.vector.tensor_scalar_add(q2T[:, sl], qp[:], bqq_sb[:])

                    for j in range(4):
                        mt = c * 4 + j
                        msl = slice(mt * P, (mt + 1) * P)
                        vp = ppj.tile([P, 512], f32r, name="vp", tag="kv",
                                      bufs=2)[:, 0:H]
                        nc.tensor.transpose(vp[:], vT[H:P, msl], id_sb[H:P, H:P])
                        nc.vector.tensor_copy(valaug[:, mt, 0:H], vp[:])

                # ---- key bias -> eb, folded into valaug (gather-gated) --
                for c in range(NCH):
                    bp = ppj.tile([P, 512], f32, name="bp", tag="kv",
                                  bufs=2)[:, 0:16]
                    for j in range(4):
                        mt = c * 4 + j
                        msl = slice(mt * P, (mt + 1) * P)
                        nc.tensor.matmul(
                            bp[:, 4 * j : 4 * j + 4], K2[:, msl], uv_col[:],
                            start=True, stop=True,
                        )
                    nc.scalar.activation(
                        ebias[:, c * 4 : (c + 1) * 4], bp[:, 0:16:4], Exp
                    )
                    for j in range(4):
                        mt = c * 4 + j
                        nc.vector.tensor_scalar_mul(
                            valaug[:, mt, :], valaug[:, mt, :],
                            ebias[:, mt : mt + 1],
                        )

            # ---- sweep 2: attention, query-chunk outer ------------------
            with (
                tc.tile_pool(name="expp", bufs=4) as exp_pool,
                tc.tile_pool(name="posb", bufs=2) as osb_pool,
                tc.tile_pool(name="pinv", bufs=2) as inv_pool,
                tc.tile_pool(name="ps_s", bufs=2, space="PSUM") as pp_s,
                tc.tile_pool(name="ps_o", bufs=2, space="PSUM") as pp_o,
            ):
                for tq in range(NTQ):
                    tsl = slice(tq * 512, (tq + 1) * 512)
                    mts = [mt for mt in range(MT) if _live(tq, mt)]
                    groups = [mts[i : i + 3] for i in range(0, len(mts), 3)]
                    oT_ps = pp_o.tile([H + 1, 512], f32, name="oT_ps")
                    n_done = 0
                    pend = []
                    for g in groups + [None, None]:
                        if g is not None:
                            sp = pp_s.tile([P, 3, 512], f32, name="sp", tag="sp")
                            for i, mt in enumerate(g):
                                msl = slice(mt * P, (mt + 1) * P)
                                diag = _is_diag(tq, mt)
                                nc.tensor.matmul(
                                    sp[:, i, :], K2[:, msl], q2T[:, tsl],
                                    start=True, stop=not diag,
                                )
                                if diag:
                                    nc.tensor.matmul(
                                        sp[:, i, :], id_sb[:, :],
                                        maskA[:, mt - tq * 4, :],
                                        start=False, stop=True,
                                    )
                            ex = exp_pool.tile([P, 3, 512], f32r, name="ex")
                            nc.scalar.activation(
                                ex[:, 0 : len(g), :], sp[:, 0 : len(g), :], Exp
                            )
                        # attnval emitted two groups late, so the next two
                        # groups' score matmuls sit ahead of it in the
                        # in-order PE queue and PE never stalls on exp
                        if g is not None:
                            pend.append((g, ex))
                        if (len(pend) > 2) or (g is None and pend):
                            pg, pex = pend.pop(0)
                            for i, mt in enumerate(pg):
                                nc.tensor.matmul(
                                    oT_ps[:], valaug[:, mt, :], pex[:, i, :],
                                    start=(n_done == 0),
                                    stop=(n_done == len(mts) - 1),
                                )
                                n_done += 1
                    oT_sb = osb_pool.tile([H + 1, 512], f32, name="oT_sb")
                    nc.vector.tensor_copy(oT_sb[:], oT_ps[:])
                    for j in range(4):
                        # share the accumulator slots (free once oT_sb is
                        # copied) instead of the score slots, which would
                        # stall the next query chunk's score matmuls
                        tp = pp_o.tile([P, H + 1], f32, name="tp", tag="oT_ps")
                        nc.tensor.transpose(
                            tp[:], oT_sb[:, j * P : (j + 1) * P],
                            id_sb[0 : H + 1, 0 : H + 1].bitcast(f32),
                        )
                        inv = inv_pool.tile([P, 1], f32, name="inv")
                        nc.vector.reciprocal(inv[:], tp[:, H : H + 1])
                        nc.vector.tensor_scalar_mul(
                            outall[:, tq * 4 + j, :], tp[:, 0:H], inv[:]
                        )
                nc.sync.dma_start(out_r, outall[:])

    nc.compile()
    return nc


def _get_module():
    if "nc" not in _CACHE:
        _CACHE["nc"] = _build_module()
    return _CACHE["nc"]


def _make_in_maps(inputs):
    inp = np.asarray(inputs["inp_data"], dtype=np.float32)
    Wq = np.asarray(inputs["Wq"], dtype=np.float32)
    bq = np.asarray(inputs["bq"], dtype=np.float32)
    Wk = np.asarray(inputs["Wk"], dtype=np.float32)
    bk = np.asarray(inputs["bk"], dtype=np.float32)
    Wv = np.asarray(inputs["Wv"], dtype=np.float32)
    bv = np.asarray(inputs["bv"], dtype=np.float32)
    Wr = np.asarray(inputs["Wr"], dtype=np.float32)
    br = np.asarray(inputs["br"], dtype=np.float32)
    u = np.asarray(inputs["u"], dtype=np.float32)
    v = np.asarray(inputs["v"], dtype=np.float32)

    if "peT" not in _CACHE:
        _CACHE["peT"] = _posenc_T()
    peT = _CACHE["peT"]
    common = {
        "wkv": np.ascontiguousarray(np.concatenate([Wk, Wv], axis=1)),
        "wqq": np.ascontiguousarray(np.concatenate([Wq, Wq], axis=1)),
        "wr2": np.ascontiguousarray(
            np.concatenate([np.zeros_like(Wr), Wr], axis=1)
        ),
        "bkv": np.ascontiguousarray(np.concatenate([bk, bv]).reshape(2 * H, 1)),
        "bqq": np.ascontiguousarray(np.concatenate([bq, bq]).reshape(2 * H, 1)),
        "br": np.ascontiguousarray(br.reshape(H, 1)),
        "u": np.ascontiguousarray(u.reshape(H, 1)),
        "v": np.ascontiguousarray(v.reshape(H, 1)),
    }
    in_maps = []
    for b in range(NCORES):
        m = {
            "xT": np.ascontiguousarray(inp[:, b, :].T),
            "peTs": np.ascontiguousarray(peT[:, b * 512 : (b + 1) * 512]),
        }
        m.update(common)
        in_maps.append(m)
    return in_maps


def _run(in_maps, trace=False):
    from concourse.bass_utils import run_bass_kernel_spmd

    nc = _get_module()
    return run_bass_kernel_spmd(
        nc, in_maps, core_ids=list(range(NCORES)), trace=trace
    )


def _timed_run(in_maps, iters=5, reps=1):
    """Replicates bass2jax.run_bass_via_pjrt's multi-core path, but keeps the
    jitted callable and device-resident inputs so repeated executions can be
    wall-clock timed (no NTFF profiling is available through the axon client).
    """
    import time

    import jax
    import concourse.mybir as mybir
    from concourse.bass2jax import (
        _bass_exec_p,
        install_neuronx_cc_hook,
        partition_id_tensor,
    )
    from jax.experimental.shard_map import shard_map
    from jax.sharding import Mesh, NamedSharding, PartitionSpec

    nc = _get_module()
    install_neuronx_cc_hook()
    partition_name = nc.partition_id_tensor.name if nc.partition_id_tensor else None

    in_names, out_names, out_avals, zero_shapes = [], [], [], []
    for alloc in nc.m.functions[0].allocations:
        if not isinstance(alloc, mybir.MemoryLocationSet):
            continue
        name = alloc.memorylocations[0].name
        if alloc.kind == "ExternalInput":
            if name != partition_name:
                in_names.append(name)
        elif alloc.kind == "ExternalOutput":
            out_names.append(name)
            shape = tuple(alloc.tensor_shape)
            dtype = mybir.dt.np(alloc.dtype)
            out_avals.append(jax.core.ShapedArray(shape, dtype))
            zero_shapes.append((shape, dtype))
    n_params = len(in_names)
    all_names = in_names + out_names
    if partition_name is not None:
        all_names = all_names + [partition_name]
    donate = tuple(range(n_params, n_params + len(out_names)))

    def _body(*args):
        operands = list(args)
        if partition_name is not None:
            operands.append(partition_id_tensor())
        outs = _bass_exec_p.bind(
            *operands,
            out_avals=tuple(out_avals),
            in_names=tuple(all_names),
            out_names=tuple(out_names),
            lowering_input_output_aliases=(),
            sim_require_finite=True,
            sim_require_nnan=True,
            nc=nc,
        )
        return tuple(outs)

    devices = jax.devices()[:NCORES]
    mesh = Mesh(np.asarray(devices), ("core",))
    spec = PartitionSpec("core")
    in_specs = (spec,) * (n_params + len(out_names))
    sharded = jax.jit(
        shard_map(
            _body, mesh=mesh, in_specs=in_specs,
            out_specs=(spec,) * len(out_names), check_rep=False,
        ),
        donate_argnums=donate,
        keep_unused=True,
    )
    sharding = NamedSharding(mesh, spec)
    concat_in = [
        jax.device_put(
            np.concatenate([in_maps[c][nm] for c in range(NCORES)], axis=0), sharding
        )
        for nm in in_names
    ]

    def zeros():
        return [
            jax.device_put(np.zeros((NCORES * s[0], *s[1:]), d), sharding)
            for (s, d) in zero_shapes
        ]

    out = sharded(*concat_in, *zeros())
    jax.block_until_ready(out)
    times = []
    for _ in range(iters):
        zs = zeros()
        jax.block_until_ready(zs)
        t0 = time.perf_counter()
        out = sharded(*concat_in, *zs)
        jax.block_until_ready(out)
        times.append(time.perf_counter() - t0)
    results = {
        nm: np.asarray(out[i]).reshape(NCORES, *out_avals[i].shape)
        for i, nm in enumerate(out_names)
    }
    return results, times


def kernel(**inputs) -> np.ndarray:
    in_maps = _make_in_maps(inputs)
    res = _run(in_maps, trace=False)
    out = np.stack([res.results[b]["out"] for b in range(NCORES)], axis=1)
    return np.ascontiguousarray(out.astype(np.float32))



# revision 12
# speedup vs baseline: 1.1647x; 1.1647x over previous
"""Trainium2 Bass kernel for nn_Attention_Module_15152644620833 (v4).

Reference computation (T=4096, B=8, D=1024, H=64, half=2048):
    q   = x[:half] @ Wq + bq            (half, B, H)
    k   = x @ Wk + bk                   (T, B, H)
    val = x @ Wv + bv                   (T, B, H)
    r   = posenc(T, D) @ Wr + br        (T, H)
    scores[b] = q[b] @ (k[b] + r).T + bias[b][None, :]
        where bias[b][m] = sum(u) * k[m,b,:].sum() + sum(v) * r[m,:].sum()
    causal mask on first `half` key positions, softmax over all T keys,
    out = attn @ val                    (half, B, H)

Sharding: data-parallel over batch, one batch per NeuronCore (8 cores).

v4 changes vs v3:
  - No AllGather (cost-model collectives have a 15us floor).  r is computed
    fully locally from a 512-column trig table peT0 plus per-chunk rotated
    weights Wr_c = R_c.T @ Wr (angle-addition identity; host folds the
    constant rotation into the weight, device does the input-dependent
    matmul).  r chunk c = Wr_c.T @ peT0.
  - Incremental attention: x chunks stream in order [0,1,4,5,6,7,2,3] and
    each (query-chunk tq, key-chunk c) score/softmax/attnval task is emitted
    as soon as its operands exist, so attention overlaps the x DMA stream
    instead of starting after it.
  - K2 rows swapped (r at 0:64, k at 64:128) so the local r matmul output
    (M=64, partitions 0:64) is a valid PSUM dst without zero-padding wrc.
  - Natural chunk order 0..7: attention tiles unlock evenly under the
    stream; four per-tq oT accumulators live concurrently (one PSUM bank
    each, one accumulation group each).
  - Per-tq output DMAs on the Pool HWDGE ring overlap the output writeback;
    peT0/wrc ride the ACT ring so they don't head-of-line-block the x
    stream on the SP ring.

Per-core device algorithm (f32r matmuls, fp32 PSUM):
    K2 (128, T):  rows 0:64 = k.T + bk, rows 64:128 = r.T + br
    q2 (128, half): rows 0:64 = q.T + bq, rows 64:128 = the same q.T
        -> scoresT(m,t) = K2[:,mtile].T @ q2 = q.k + q.r   (K=128)
    softmax key bias folded multiplicatively into val:
        exp(s + bias[m]) = exp(s)*eb[m]; eb scales both the val columns and
        the ones column (denominator), so attn is unchanged (exact).
    causal mask: accumulate identity.T @ maskA (-1e30) into scores PSUM of
        diagonal tiles; fully-masked tiles are skipped.
    expT = exp(scoresT)  (no max subtraction: |scores| < ~60)
    oT[tq] (65, 512) += valaug[mt].T @ expT  per query chunk (col 64 of
        valaug = eb -> row 64 of oT = softmax denominator)
    out (128, 64) = transpose(oT) * (1/denominator)
"""

import math

import numpy as np

T, B, D, H = 4096, 8, 1024, 64
HALF = T // 2
P = 128
DC = D // P          # 8 d-chunks
NCH = T // 512       # 8 key chunks of 512
NTQ = HALF // 512    # 4 query chunks of 512
MT = T // P          # 32 key tiles of 128
NCORES = 8

# natural stream order: attention tiles unlock evenly under the x stream
CORDER = [0, 1, 2, 3, 4, 5, 6, 7]

_CACHE = {}


def _posenc_tables():
    """peT0 (D, 512) fine trig table and the per-chunk rotation (A, B)."""
    pos = np.arange(512, dtype=np.float32)[:, None]
    div = np.exp(
        (np.arange(0, D, 2, dtype=np.float32)
         * np.float32(-(math.log(10000.0) / D))).astype(np.float32)
    ).astype(np.float32)
    ang = (pos * div).astype(np.float32)
    pe0 = np.stack([np.sin(ang), np.cos(ang)], axis=-1).reshape(512, D)
    return np.ascontiguousarray(pe0.T), div.astype(np.float64)


def _live(tq, c):
    """Key chunk c contributes to query chunk tq (not fully masked)."""
    return c >= 4 or c <= tq


def _build_module():
    import concourse.bacc as bacc
    import concourse.bass_isa as bass_isa
    import concourse.mybir as mybir
    from concourse.masks import make_identity
    from concourse.tile import TileContext

    f32 = mybir.dt.float32
    f32r = mybir.dt.float32r
    bf16 = mybir.dt.bfloat16
    Exp = mybir.ActivationFunctionType.Exp

    nc = bacc.Bacc(num_devices=NCORES)

    xT_h = nc.dram_tensor("xT", [D, T], f32r, kind="ExternalInput")
    peT0_h = nc.dram_tensor("peT0", [D, 512], f32r, kind="ExternalInput")
    wrc_h = nc.dram_tensor("wrc", [D, NCH * H], f32r, kind="ExternalInput")
    wkv_h = nc.dram_tensor("wkv", [D, 2 * H], f32r, kind="ExternalInput")
    wqq_h = nc.dram_tensor("wqq", [D, 2 * H], f32r, kind="ExternalInput")
    bkv_h = nc.dram_tensor("bkv", [2 * H, 1], f32, kind="ExternalInput")
    bqq_h = nc.dram_tensor("bqq", [2 * H, 1], f32, kind="ExternalInput")
    br_h = nc.dram_tensor("br", [H, 1], f32, kind="ExternalInput")
    u_h = nc.dram_tensor("u", [H, 1], f32, kind="ExternalInput")
    v_h = nc.dram_tensor("v", [H, 1], f32, kind="ExternalInput")
    out_h = nc.dram_tensor("out", [HALF, H], f32, kind="ExternalOutput")

    xT_r = xT_h[:, :].rearrange("(c p) t -> p c t", p=P)       # (128, 8, T)
    peT0_r = peT0_h[:, :].rearrange("(c p) t -> p c t", p=P)   # (128, 8, 512)
    wrc_r = wrc_h[:, :].rearrange("(k p) (c h) -> p k c h", p=P, c=NCH)
    wkv_r = wkv_h[:, :].rearrange("(c p) h -> p c h", p=P)
    wqq_r = wqq_h[:, :].rearrange("(c p) h -> p c h", p=P)
    out_r = out_h[:, :].rearrange("(g p) h -> p g h", p=P)     # (128, 16, 64)

    with TileContext(nc) as tc, tc.tile_pool(name="persist", bufs=1) as persist:

        def _tile(shape, name, dt=f32):
            return persist.tile(shape, dt, name=name)

        # ---- persistent SBUF tiles -------------------------------------
        wkv_sb = _tile([P, DC, 2 * H], "wkv_sb", f32r)
        wqq_sb = _tile([P, DC, 2 * H], "wqq_sb", f32r)
        wrc_sb = _tile([P, DC, NCH, H], "wrc_sb", f32r)
        peT0_sb = _tile([P, DC, 512], "peT0_sb", f32r)
        bkv_sb = _tile([2 * H, 1], "bkv_sb")
        bqq_sb = _tile([2 * H, 1], "bqq_sb")
        brc_sb = _tile([2 * H, 1], "brc_sb")    # br loaded at rows 64:128
        u_cl = _tile([H, 1], "u_cl")
        v_cl = _tile([H, 1], "v_cl")
        u_all = _tile([H, 1], "u_all")
        v_all = _tile([H, 1], "v_all")
        uvf = _tile([2 * H, 1], "uvf")
        uv_col = _tile([2 * H, 4], "uv_col", f32r)
        id_sb = _tile([P, P], "id_sb", f32r)
        maskA = _tile([P, 4, 512], "maskA", f32r)
        K2 = _tile([P, T], "K2", f32r)          # 0:64 r.T+br, 64:128 k.T+bk
        vT = _tile([H, T], "vT", f32r)          # v.T+bv
        q2T = _tile([P, HALF], "q2T", f32r)     # rows 0:64 and 64:128 = q.T
        valaug = _tile([P, MT, H + 1], "valaug", f32r)

        # ---- small-constant DMAs (SP ring) -----------------------------
        nc.sync.dma_start(wkv_sb[:], wkv_r)
        nc.sync.dma_start(wqq_sb[:], wqq_r)
        nc.sync.dma_start(bkv_sb[:], bkv_h[:, :])
        nc.sync.dma_start(bqq_sb[:], bqq_h[:, :])
        nc.sync.dma_start(brc_sb[0:H, :], br_h[:, :])
        nc.sync.dma_start(u_cl[:], u_h[:, :])
        nc.sync.dma_start(v_cl[:], v_h[:, :])
        # ACT HWDGE ring: contends with (instead of queueing behind) the
        # SP-ring x chunk stream
        nc.scalar.dma_start(peT0_sb[:], peT0_r)
        nc.scalar.dma_start(wrc_sb[:], wrc_r)

        # f32r tiles cannot be produced by memset/affine_select directly;
        # build constants in f32 scratch, cast-copy on DVE.
        with tc.tile_pool(name="setupf", bufs=1) as setupf:
            idf = setupf.tile([P, P], f32, name="idf")
            make_identity(nc, idf[:])
            nc.vector.tensor_copy(id_sb[:], idf[:])
            maskAf = setupf.tile([P, 4, 512], f32, name="maskAf")
            nc.gpsimd.memset(maskAf[:], 0.0)
            for rel in range(4):
                nc.gpsimd.affine_select(
                    out=maskAf[:, rel, :], in_=maskAf[:, rel, :],
                    compare_op=mybir.AluOpType.is_ge, fill=-1e30,
                    base=-P * rel, pattern=[[1, 512]], channel_multiplier=-1,
                )
            nc.vector.tensor_copy(maskA[:], maskAf[:])
            onesf = setupf.tile([P, MT], f32, name="onesf")
            nc.gpsimd.memset(onesf[:], 1.0)
            nc.vector.tensor_copy(valaug[:, :, H], onesf[:, 0:MT])

        # sum(u) broadcast to rows 0:64, sum(v) to rows 64:128 of uv_col
        nc.gpsimd.partition_all_reduce(u_all[:], u_cl[:], H, bass_isa.ReduceOp.add)
        nc.gpsimd.partition_all_reduce(v_all[:], v_cl[:], H, bass_isa.ReduceOp.add)
        nc.vector.tensor_copy(uvf[0:H, :], v_all[:])
        nc.sync.dma_start(uvf[H : 2 * H, :], u_all[:])  # partition shift
        nc.vector.tensor_copy(uv_col[:], uvf[:, 0:1].to_broadcast((2 * H, 4)))

        with (
            tc.tile_pool(name="xstream", bufs=2) as xpool,
            tc.tile_pool(name="expp", bufs=6) as exp_pool,
            tc.tile_pool(name="posb", bufs=2) as osb_pool,
            tc.tile_pool(name="ps_s", bufs=2, space="PSUM") as pool_s,   # 2 banks
            tc.tile_pool(name="ps_o", bufs=4, space="PSUM") as pool_o,   # 4 banks
            tc.tile_pool(name="ps_c", bufs=2, space="PSUM") as pool_c,   # 2 banks
        ):
            # kick off the x chunk stream right away (SP ring, in CORDER)
            xts = {}
            for c in CORDER:
                xt = xpool.tile([P, DC, 512], f32r, name=f"xt{c}", tag="xt")
                nc.sync.dma_start(xt[:], xT_r[:, :, c * 512 : (c + 1) * 512])
                xts[c] = xt

            # ---- attention task machinery ------------------------------
            oT = {}           # tq -> PSUM accumulator tile (H+1, 512)
            n_done = {tq: 0 for tq in range(NTQ)}
            n_live = {tq: 4 * sum(_live(tq, c) for c in range(NCH))
                      for tq in range(NTQ)}
            pend = []         # FIFO of (tq, mt, ex) tiles awaiting attnval

            def emit_attnval(tq, mt, ex):
                nc.tensor.matmul(
                    oT[tq][:], valaug[:, mt, :], ex[:],
                    start=(n_done[tq] == 0),
                    stop=(n_done[tq] == n_live[tq] - 1),
                )
                n_done[tq] += 1
                if n_done[tq] == n_live[tq]:
                    # finish tq: copy accumulator out, transpose, divide by
                    # the denominator row, DMA out on the Pool ring
                    oT_sb = osb_pool.tile([H + 1, 512], f32, name="oT_sb",
                                          tag="osb")
                    nc.vector.tensor_copy(oT_sb[:], oT[tq][:])
                    osc = osb_pool.tile([P, 4, H], f32, name="osc", tag="osc")
                    for j in range(4):
                        tp = pool_c.tile([P, 512], f32, name="tp", tag="pc")
                        nc.tensor.transpose(
                            tp[:, 0 : H + 1], oT_sb[:, j * P : (j + 1) * P],
                            id_sb[0 : H + 1, 0 : H + 1].bitcast(f32),
                        )
                        inv = osb_pool.tile([P, 1], f32, name="inv", tag="inv")
                        nc.vector.reciprocal(inv[:], tp[:, H : H + 1])
                        nc.vector.tensor_scalar_mul(
                            osc[:, j, :], tp[:, 0:H], inv[:]
                        )
                    nc.gpsimd.dma_start(
                        out_r[:, tq * 4 : (tq + 1) * 4, :], osc[:]
                    )

            def emit_scores(tq, c):
                tsl = slice(tq * 512, (tq + 1) * 512)
                diag = c == tq
                for j in range(4):
                    mt = 4 * c + j
                    msl = slice(mt * P, (mt + 1) * P)
                    sp = pool_s.tile([P, 512], f32, name="sp", tag="ps")
                    nc.tensor.matmul(
                        sp[:], K2[:, msl], q2T[:, tsl],
                        start=True, stop=not diag,
                    )
                    if diag:
                        nc.tensor.matmul(
                            sp[:], id_sb[:, :], maskA[:, j, :],
                            start=False, stop=True,
                        )
                    ex = exp_pool.tile([P, 512], f32r, name="ex", tag="ex")
                    nc.scalar.activation(ex[:], sp[:], Exp)
                    pend.append((tq, mt, ex))
                    # attnval lags so the in-order PE queue always has score
                    # matmuls ahead of the exp result it waits on
                    while len(pend) > 3:
                        emit_attnval(*pend.pop(0))

            # ---- main chunk loop ---------------------------------------
            ready_q = []
            ready_k = []
            for c in CORDER:
                csl = slice(c * 512, (c + 1) * 512)
                xt = xts[c]

                # r chunk c: out (64, 512) at partitions 0:64 -> K2 rows 0:64
                spr = pool_c.tile([P, 512], f32, name="spr", tag="pc")
                for dc in range(DC):
                    nc.tensor.matmul(
                        spr[0:H, :], wrc_sb[:, dc, c, :], peT0_sb[:, dc, :],
                        start=(dc == 0), stop=(dc == DC - 1),
                    )
                nc.vector.tensor_scalar_add(
                    K2[0:H, csl], spr[0:H, :], brc_sb[0:H, :]
                )

                # k/v projections for chunk c ([Wv | Wk] stationary)
                kvp = pool_c.tile([P, 512], f32, name="kvp", tag="pc")
                for dc in range(DC):
                    nc.tensor.matmul(
                        kvp[:], wkv_sb[:, dc, :], xt[:, dc, :],
                        start=(dc == 0), stop=(dc == DC - 1),
                    )
                nc.vector.tensor_scalar_add(
                    vT[:, csl], kvp[0:H, :], bkv_sb[0:H, :]
                )
                nc.vector.tensor_scalar_add(
                    K2[H:P, csl], kvp[H:P, :], bkv_sb[H : 2 * H, :]
                )

                # q projection for chunks 0..3
                if c < NTQ:
                    qp = pool_c.tile([P, 512], f32, name="qp", tag="pc")
                    for dc in range(DC):
                        nc.tensor.matmul(
                            qp[:], wqq_sb[:, dc, :], xt[:, dc, :],
                            start=(dc == 0), stop=(dc == DC - 1),
                        )
                    nc.vector.tensor_scalar_add(q2T[:, csl], qp[:], bqq_sb[:])

                # val transposes + key bias eb folded into valaug
                vb = pool_c.tile([P, 512], f32, name="vb", tag="pc")
                ebias = osb_pool.tile([P, 4], f32, name="ebias", tag="eb")
                for j in range(4):
                    mt = c * 4 + j
                    msl = slice(mt * P, (mt + 1) * P)
                    nc.tensor.transpose(
                        vb[:, 64 * j : 64 * j + 64].bitcast(f32r),
                        vT[:, msl], id_sb[0:H, 0:H],
                    )
                    nc.tensor.matmul(
                        vb[:, 256 + 4 * j : 260 + 4 * j], K2[:, msl], uv_col[:],
                        start=True, stop=True,
                    )
                nc.scalar.activation(ebias[:], vb[:, 256:272:4], Exp)
                for j in range(4):
                    mt = c * 4 + j
                    nc.vector.tensor_copy(
                        valaug[:, mt, 0:H],
                        vb[:, 64 * j : 64 * j + 64].bitcast(f32r),
                    )
                for j in range(4):
                    mt = c * 4 + j
                    nc.vector.tensor_scalar_mul(
                        valaug[:, mt, :], valaug[:, mt, :], ebias[:, j : j + 1]
                    )

                # newly unlocked attention tasks
                if c < NTQ:
                    ready_q.append(c)
                prev_k = list(ready_k)
                ready_k.append(c)
                for tq in ready_q:
                    if tq not in oT and any(_live(tq, ck) for ck in ready_k):
                        oT[tq] = pool_o.tile([H + 1, 512], f32,
                                             name=f"oT_{tq}", tag="oT")
                if c < NTQ:
                    # newly-ready query chunk: all previously-landed key chunks
                    for ck in prev_k:
                        if _live(c, ck):
                            emit_scores(c, ck)
                # newly-landed key chunk: all ready query chunks
                for tq in ready_q:
                    if _live(tq, c):
                        emit_scores(tq, c)

            while pend:
                emit_attnval(*pend.pop(0))

    nc.compile()
    return nc


def _get_module():
    if "nc" not in _CACHE:
        _CACHE["nc"] = _build_module()
    return _CACHE["nc"]


def _make_in_maps(inputs):
    inp = np.asarray(inputs["inp_data"], dtype=np.float32)
    Wq = np.asarray(inputs["Wq"], dtype=np.float32)
    bq = np.asarray(inputs["bq"], dtype=np.float32)
    Wk = np.asarray(inputs["Wk"], dtype=np.float32)
    bk = np.asarray(inputs["bk"], dtype=np.float32)
    Wv = np.asarray(inputs["Wv"], dtype=np.float32)
    bv = np.asarray(inputs["bv"], dtype=np.float32)
    Wr = np.asarray(inputs["Wr"], dtype=np.float32)
    br = np.asarray(inputs["br"], dtype=np.float32)
    u = np.asarray(inputs["u"], dtype=np.float32)
    v = np.asarray(inputs["v"], dtype=np.float32)

    if "peT0" not in _CACHE:
        _CACHE["peT0"], _CACHE["omega"] = _posenc_tables()
    peT0 = _CACHE["peT0"]
    omega = _CACHE["omega"]

    # rotated weights: r chunk c = (R_c.T @ Wr).T-contract against peT0
    wrc = np.empty((D, NCH * H), dtype=np.float32)
    for c in range(NCH):
        A = np.sin(512 * c * omega)
        B = np.cos(512 * c * omega)
        wc = np.empty_like(Wr)
        wc[0::2] = (B[:, None] * Wr[0::2] - A[:, None] * Wr[1::2]).astype(np.float32)
        wc[1::2] = (A[:, None] * Wr[0::2] + B[:, None] * Wr[1::2]).astype(np.float32)
        wrc[:, c * H : (c + 1) * H] = wc

    common = {
        "peT0": np.ascontiguousarray(peT0),
        "wrc": np.ascontiguousarray(wrc),
        "wkv": np.ascontiguousarray(np.concatenate([Wv, Wk], axis=1)),
        "wqq": np.ascontiguousarray(np.concatenate([Wq, Wq], axis=1)),
        "bkv": np.ascontiguousarray(np.concatenate([bv, bk]).reshape(2 * H, 1)),
        "bqq": np.ascontiguousarray(np.concatenate([bq, bq]).reshape(2 * H, 1)),
        "br": np.ascontiguousarray(br.reshape(H, 1)),
        "u": np.ascontiguousarray(u.reshape(H, 1)),
        "v": np.ascontiguousarray(v.reshape(H, 1)),
    }
    in_maps = []
    for b in range(NCORES):
        m = {"xT": np.ascontiguousarray(inp[:, b, :].T)}
        m.update(common)
        in_maps.append(m)
    return in_maps


def _run(in_maps, trace=False):
    from concourse.bass_utils import run_bass_kernel_spmd

    nc = _get_module()
    return run_bass_kernel_spmd(
        nc, in_maps, core_ids=list(range(NCORES)), trace=trace
    )


def _timed_run(in_maps, iters=5, reps=1):
    """Replicates bass2jax.run_bass_via_pjrt's multi-core path, but keeps the
    jitted callable and device-resident inputs so repeated executions can be
    wall-clock timed (no NTFF profiling is available through the axon client).
    """
    import time

    import jax
    import concourse.mybir as mybir
    from concourse.bass2jax import (
        _bass_exec_p,
        install_neuronx_cc_hook,
        partition_id_tensor,
    )
    from jax.experimental.shard_map import shard_map
    from jax.sharding import Mesh, NamedSharding, PartitionSpec

    nc = _get_module()
    install_neuronx_cc_hook()
    partition_name = nc.partition_id_tensor.name if nc.partition_id_tensor else None

    in_names, out_names, out_avals, zero_shapes = [], [], [], []
    for alloc in nc.m.functions[0].allocations:
        if not isinstance(alloc, mybir.MemoryLocationSet):
            continue
        name = alloc.memorylocations[0].name
        if alloc.kind == "ExternalInput":
            if name != partition_name:
                in_names.append(name)
        elif alloc.kind == "ExternalOutput":
            out_names.append(name)
            shape = tuple(alloc.tensor_shape)
            dtype = mybir.dt.np(alloc.dtype)
            out_avals.append(jax.core.ShapedArray(shape, dtype))
            zero_shapes.append((shape, dtype))
    n_params = len(in_names)
    all_names = in_names + out_names
    if partition_name is not None:
        all_names = all_names + [partition_name]
    donate = tuple(range(n_params, n_params + len(out_names)))

    def _body(*args):
        operands = list(args)
        if partition_name is not None:
            operands.append(partition_id_tensor())
        outs = _bass_exec_p.bind(
            *operands,
            out_avals=tuple(out_avals),
            in_names=tuple(all_names),
            out_names=tuple(out_names),
            lowering_input_output_aliases=(),
            sim_require_finite=True,
            sim_require_nnan=True,
            nc=nc,
        )
        return tuple(outs)

    devices = jax.devices()[:NCORES]
    mesh = Mesh(np.asarray(devices), ("core",))
    spec = PartitionSpec("core")
    in_specs = (spec,) * (n_params + len(out_names))
    sharded = jax.jit(
        shard_map(
            _body, mesh=mesh, in_specs=in_specs,
            out_specs=(spec,) * len(out_names), check_rep=False,
        ),
        donate_argnums=donate,
        keep_unused=True,
    )
    sharding = NamedSharding(mesh, spec)
    concat_in = [
        jax.device_put(
            np.concatenate([in_maps[c][nm] for c in range(NCORES)], axis=0), sharding
        )
        for nm in in_names
    ]

    def zeros():
        return [
            jax.device_put(np.zeros((NCORES * s[0], *s[1:]), d), sharding)
            for (s, d) in zero_shapes
        ]

    out = sharded(*concat_in, *zeros())
    jax.block_until_ready(out)
    times = []
    for _ in range(iters):
        zs = zeros()
        jax.block_until_ready(zs)
        t0 = time.perf_counter()
        out = sharded(*concat_in, *zs)
        jax.block_until_ready(out)
        times.append(time.perf_counter() - t0)
    results = {
        nm: np.asarray(out[i]).reshape(NCORES, *out_avals[i].shape)
        for i, nm in enumerate(out_names)
    }
    return results, times


def kernel(**inputs) -> np.ndarray:
    in_maps = _make_in_maps(inputs)
    res = _run(in_maps, trace=False)
    out = np.stack([res.results[b]["out"] for b in range(NCORES)], axis=1)
    return np.ascontiguousarray(out.astype(np.float32))
